# revision 1
# baseline (speedup 1.0000x reference)
"""Trainium2 Bass kernel: batch-independent contrastive loss (SupCon-style with
EMA-normalized negatives).

Math (derived from the reference):
  CF = concat(views) [N=4096, D=256], S = CF @ CF.T / T
  Each row i has exactly one positive p(i) = (i+B) mod N; neg_mask keeps the
  diagonal.  With m_i = row max = ||f_i||^2/T:
    Z_i  = sum_j exp(S_ij - m_i)            = e^{-m_i} * P_i,  P_i = sum_j exp(S_ij)
    W_i  = sum_j exp(S_ij - m_i)(S_ij-m_i)  = e^{-m_i} * (Q_i - m_i P_i),
           Q_i = sum_j exp(S_ij) S_ij
    Zneg_i = Z_i - e_pos_i,  Wneg_i = W_i - e_pos_i * Lpos_i
    u_new  = (1-g) u[idx] + g Zneg   (view-0 rows)
    loss_i = Wneg_i / u_new_{i mod B} - Lpos_i ;  output = mean_i loss_i

Sharding: by sample across 8 cores (each core owns 256 samples = 512 anchor
rows covering both views), so the u_new coupling between row b and b+B stays
on-core.  The contrast side (all 4096 columns) is replicated.  Per core:
one bf16 matmul chain builds S row-blocks in PSUM; the Scalar engine does
exp with free-dim accumulation (P), the Vector engine does the fused
multiply-reduce (Q); a final ~25 tiny [128,4] vector ops assemble the loss.
Host only gathers u at 2048 indices, preps bf16 layouts, and averages the
8x[128,4] per-row outputs.
"""

import numpy as np
import ml_dtypes

GAMMA = 0.9
TEMP = 0.07
B, V, D = 2048, 2, 256
N = B * V            # 4096 contrast rows/cols
NCORES = 8
SPC = B // NCORES    # 256 samples per core
RPC = V * SPC        # 512 anchor rows per core
RC = RPC // 128      # 4 chunks of 128 anchor rows (0,1: view0; 2,3: view1)
JT = 1024            # contrast-column tile (2 PSUM banks)
NJT = N // JT

_CACHE = {}


def _build_module():
    import concourse.bacc as bacc
    import concourse.tile as tile
    from concourse import mybir

    f32 = mybir.dt.float32
    bf16 = mybir.dt.bfloat16
    AF = mybir.ActivationFunctionType
    ALU = mybir.AluOpType
    AX = mybir.AxisListType

    nc = bacc.Bacc(
        "TRN2", target_bir_lowering=False, debug=False, enable_asserts=False
    )
    # DMA has ~1us fixed cost per dma_start and executes FIFO per issuing
    # engine, with all rings sharing ~300GB/s of HBM read bandwidth; the
    # arrangement below (anc first on sync, ct pieces alternating between
    # the scalar HWDGE and gpsimd SWDGE rings) measured fastest.
    anc_d = nc.dram_tensor("anc", [128, 2 * RPC], bf16, kind="ExternalInput")
    fa_d = nc.dram_tensor("fa", [128, RC * D], bf16, kind="ExternalInput")
    ug_d = nc.dram_tensor("ug", [128, 2], f32, kind="ExternalInput")  # (1-g)*u[idx]
    # ct as 8 column-pieces [128, k0-block | k1-block]; piece i covers
    # columns [i*512, (i+1)*512) of the contrast side, so compute can
    # chase the DMA stream piece by piece.
    NPC = N // 512                       # 8 pieces
    ct_d = nc.dram_tensor("ct", [NPC, 128, 2 * 512], bf16, kind="ExternalInput")
    out_d = nc.dram_tensor("loss_rows", [128, RC], f32, kind="ExternalOutput")

    with tile.TileContext(nc) as tc:
        with tc.tile_pool(name="singles", bufs=1) as singles, \
             tc.tile_pool(name="psum", bufs=4, space="PSUM") as psum_pool, \
             tc.tile_pool(name="work", bufs=3) as work, \
             tc.tile_pool(name="stats", bufs=1) as stats:
            # The three DGE rings share ~300GB/s of HBM read bandwidth, so
            # the BYTE order across rings must match consumption order:
            # anc first (gates every matmul), then ct pieces round-robined
            # so pieces complete roughly in index order; fa/ug (tail-only
            # consumers) last.
            anc_flat = singles.tile([128, 2 * RPC], bf16)
            nc.sync.dma_start(out=anc_flat, in_=anc_d[:, :])
            ct_pc = [None] * NPC
            for i in range(NPC):
                t = singles.tile([128, 2 * 512], bf16, tag=f"ct_{i}")
                eng = nc.scalar if i % 2 == 0 else nc.gpsimd
                eng.dma_start(out=t, in_=ct_d[i])
                ct_pc[i] = t
            fa_flat = singles.tile([128, RC * D], bf16)
            nc.sync.dma_start(out=fa_flat, in_=fa_d[:, :])
            ug_sb = singles.tile([128, 2], f32)
            nc.sync.dma_start(out=ug_sb, in_=ug_d[:, :])
            anc_sb = anc_flat.rearrange("p (k r) -> p k r", k=2)
            fa_sb = fa_flat.rearrange("p (rc d) -> p rc d", rc=RC)

            # PE warmup: dependency-free dummy matmuls (on a memset tile)
            # keep the tensor clock ungated while the inputs stream in.
            warm_sb = singles.tile([128, 512], bf16)
            nc.vector.memset(warm_sb, 0.0)
            wps = psum_pool.tile([128, JT], f32, tag="ps")
            for w in range(8):
                nc.tensor.matmul(
                    wps[:, 0:512],
                    lhsT=warm_sb[:, 0:128],
                    rhs=warm_sb,
                    start=True, stop=True,
                )

            # ---- per-row statistics that only need the anchor features ----
            # (emitted first so they run during the ct DMA head)
            msum = stats.tile([128, RC], f32)   # ||f_r||^2 (pre 1/T)
            for rc in range(RC):
                scr2 = work.tile([128, D], f32, tag="scr2")
                nc.vector.scalar_tensor_tensor(
                    out=scr2, in0=fa_sb[:, rc, :], scalar=1.0,
                    in1=fa_sb[:, rc, :], op0=ALU.mult, op1=ALU.mult,
                    accum_out=msum[:, rc:rc + 1],
                )
            pd = stats.tile([128, 2], f32)      # f_view0 . f_view1 per sample
            for s in range(2):
                scr2 = work.tile([128, D], f32, tag="scr2")
                nc.vector.scalar_tensor_tensor(
                    out=scr2, in0=fa_sb[:, s, :], scalar=1.0,
                    in1=fa_sb[:, 2 + s, :], op0=ALU.mult, op1=ALU.mult,
                    accum_out=pd[:, s:s + 1],
                )
            m4 = stats.tile([128, RC], f32)     # m = msum/T
            nc.vector.tensor_scalar_mul(m4, msum, 1.0 / TEMP)
            em = stats.tile([128, RC], f32)     # e^{-m}
            nc.scalar.activation(out=em, in_=msum, func=AF.Exp, scale=-1.0 / TEMP)
            pd4 = stats.tile([128, RC], f32)
            nc.vector.tensor_copy(pd4[:, 0:2], pd)
            nc.vector.tensor_copy(pd4[:, 2:4], pd)
            lp2 = stats.tile([128, RC], f32)    # Lpos = pd/T - m
            nc.vector.scalar_tensor_tensor(
                out=lp2, in0=pd4, scalar=1.0 / TEMP, in1=m4,
                op0=ALU.mult, op1=ALU.subtract)
            ep = stats.tile([128, RC], f32)     # e_pos
            nc.scalar.activation(out=ep, in_=lp2, func=AF.Exp)
            epl = stats.tile([128, RC], f32)
            nc.vector.tensor_mul(epl, ep, lp2)

            pacc = stats.tile([128, RC, NJT], f32)
            qacc = stats.tile([128, RC, NJT], f32)

            # jt-outer: the first RC units all consume ct pieces 0-1, giving
            # the piece DMA stream maximal slack to stay ahead of compute
            for jt in range(NJT):
                for rc in range(RC):
                    ps = psum_pool.tile([128, JT], f32, tag="ps")
                    for jb in range(JT // 512):
                        pc = ct_pc[jt * (JT // 512) + jb]
                        for k in range(2):
                            nc.tensor.matmul(
                                ps[:, jb * 512:(jb + 1) * 512],
                                lhsT=anc_sb[:, k, rc * 128:(rc + 1) * 128],
                                rhs=pc[:, k * 512:(k + 1) * 512],
                                start=(k == 0),
                                stop=(k == 1),
                            )
                    e_t = work.tile([128, JT], f32, tag="e")
                    nc.scalar.activation(
                        out=e_t, in_=ps, func=AF.Exp, scale=1.0 / TEMP,
                        accum_out=pacc[:, rc, jt:jt + 1],
                    )
                    scr = work.tile([128, JT], f32, tag="scr")
                    nc.vector.scalar_tensor_tensor(
                        out=scr, in0=e_t, scalar=1.0 / TEMP, in1=ps,
                        op0=ALU.mult, op1=ALU.mult,
                        accum_out=qacc[:, rc, jt:jt + 1],
                    )

            # ---- combine ----
            p4 = stats.tile([128, RC], f32)
            nc.vector.reduce_sum(out=p4, in_=pacc, axis=AX.X)
            q4 = stats.tile([128, RC], f32)
            nc.vector.reduce_sum(out=q4, in_=qacc, axis=AX.X)

            t2 = stats.tile([128, RC], f32)
            nc.vector.tensor_mul(t2, m4, p4)
            t3 = stats.tile([128, RC], f32)     # q4 - t2
            nc.vector.scalar_tensor_tensor(
                out=t3, in0=t2, scalar=-1.0, in1=q4,
                op0=ALU.mult, op1=ALU.add)
            w4 = stats.tile([128, RC], f32)
            nc.vector.tensor_mul(w4, em, t3)
            wn = stats.tile([128, RC], f32)     # w4 - epl
            nc.vector.scalar_tensor_tensor(
                out=wn, in0=epl, scalar=-1.0, in1=w4,
                op0=ALU.mult, op1=ALU.add)

            # u_new only needs view-0 rows (columns 0-1), so the z-path is
            # computed at [128,2] width and c4 applies 1/u_new via two
            # slice-multiplies (no ru4 broadcast copies)
            z2 = stats.tile([128, 2], f32)
            nc.vector.tensor_mul(z2, em[:, 0:2], p4[:, 0:2])
            zn2 = stats.tile([128, 2], f32)     # z2 - ep
            nc.vector.scalar_tensor_tensor(
                out=zn2, in0=ep[:, 0:2], scalar=-1.0, in1=z2,
                op0=ALU.mult, op1=ALU.add)
            un = stats.tile([128, 2], f32)      # g*zneg + (1-g)*u[idx]
            nc.vector.scalar_tensor_tensor(
                out=un, in0=zn2, scalar=GAMMA, in1=ug_sb,
                op0=ALU.mult, op1=ALU.add)
            ru = stats.tile([128, 2], f32)
            nc.vector.reciprocal(ru, un)
            c4 = stats.tile([128, RC], f32)
            nc.vector.tensor_mul(c4[:, 0:2], wn[:, 0:2], ru)
            nc.vector.tensor_mul(c4[:, 2:4], wn[:, 2:4], ru)
            out_sb = stats.tile([128, RC], f32)
            nc.vector.tensor_sub(out_sb, c4, lp2)
            nc.sync.dma_start(out=out_d[:, :], in_=out_sb)

    nc.compile()
    return nc


def _get_module():
    if "nc" not in _CACHE:
        _CACHE["nc"] = _build_module()
    return _CACHE["nc"]


def _prep_inputs(index, features, u):
    feats = np.asarray(features, dtype=np.float32)
    idx = np.asarray(index).astype(np.int64).reshape(-1)
    u_np = np.asarray(u, dtype=np.float32).reshape(-1)

    cf = np.ascontiguousarray(feats.transpose(1, 0, 2).reshape(N, D))
    cfb = cf.astype(ml_dtypes.bfloat16)
    ct = np.ascontiguousarray(cfb.T)                       # [D, N] bf16
    # [piece, 128, k0-block | k1-block]: piece i = columns [i*512,(i+1)*512)
    ct_in = np.ascontiguousarray(
        ct.reshape(2, 128, N // 512, 512).transpose(2, 1, 0, 3)
        .reshape(N // 512, 128, 2 * 512))

    in_maps = []
    for c in range(NCORES):
        rows = np.concatenate([
            np.arange(c * SPC, (c + 1) * SPC),
            np.arange(B + c * SPC, B + (c + 1) * SPC),
        ])
        anc_r = np.ascontiguousarray(ct[:, rows])          # [128*2(k), RPC]
        fa_r = cfb[rows, :]                                # [RPC, D]
        anc = np.empty((128, 2 * RPC), dtype=ml_dtypes.bfloat16)
        anc[:, 0:RPC] = anc_r[0:128]
        anc[:, RPC:2 * RPC] = anc_r[128:256]
        fa = np.empty((128, RC * D), dtype=ml_dtypes.bfloat16)
        for rc in range(RC):
            fa[:, rc * D:(rc + 1) * D] = fa_r[rc * 128:(rc + 1) * 128]
        ug_vals = (1.0 - GAMMA) * u_np[idx[c * SPC:(c + 1) * SPC]]
        ug = np.ascontiguousarray(ug_vals.reshape(2, 128).T)  # [128, 2]
        in_maps.append({"anc": anc, "fa": fa, "ug": ug, "ct": ct_in})
    return in_maps


def _run(in_maps, trace=False, **kw):
    from concourse.bass_utils import run_bass_kernel_spmd

    nc = _get_module()
    return run_bass_kernel_spmd(
        nc, in_maps, core_ids=list(range(NCORES)), trace=trace, **kw
    )


def kernel(index, features, u):
    in_maps = _prep_inputs(index, features, u)
    res = _run(in_maps)
    total = 0.0
    for c in range(NCORES):
        total += np.asarray(res.results[c]["loss_rows"], dtype=np.float64).sum()
    return np.float32(total / N)

